# revision 24
# baseline (speedup 1.0000x reference)
"""Trainium2 Bass kernel for the aux-attention module.

reference (per batch b):
    inputs = concat([enc[b], broadcast(hs[b])], -1)          # (S, 4096)
    hidden = tanh(inputs @ W1.T + b1)                        # (S, 1024)
    e      = hidden @ w2.T                                   # (S,)
    alpha  = softmax(e)
    ctx    = alpha @ enc[b]                                  # (3072,)
    out[b] = ctx @ W3.T + b3                                 # (1024,)

Strategy: data-parallel over batch (4 batches/core x 8 cores), weights
replicated. All PE matmuls in fp16 (fp32 PSUM accumulation). Softmax without
max-subtraction: w = exp(e - 4) unnormalized (e is O(1) for this model), the
1/sum(w) normalization is folded into the final output scaling.

Per core, per 128-row tile (single pass over enc, f-major layout from host):
  - hidden = tanh(enc_tile @ W1e.T + hb) on PE (25 N=512 matmuls) + ACT
  - e column via one fused DVE multiply+accumulate against broadcast w2
  - e -> row (PE transpose), w = exp(e-4) (ACT, also accumulates l), w
    broadcast across partitions (K=1 matmul), then ctx_partial[f-chunk] =
    sum_s w[s]*enc[f, s] as a DVE multiply + per-chunk reduce on the same
    f-major tile already in SBUF (no second HBM read of enc).
Tail: inv_l via reduce+reciprocal, out = (ctxT @ W3.T) * inv_l + b3.
"""

import numpy as np

import concourse.bass as bass
import concourse.tile as tile
from concourse import mybir
from concourse.bass import ds
from concourse import bass_utils

# ---------------------------------------------------------------------------
# Walrus in this container caps sync waits per instruction (one; two for
# EventSemaphore). Tile's tail drain carries one wait per live semaphore and
# Tile occasionally leaks multi-wait instructions; split extras onto cheap
# carriers.
from concourse import tile as _tile_mod
from concourse import mybir as _mybir


def _patched_drain_and_barrier(self, tick_clock, wait_clock):
    nc = self.nc
    drain_inst = nc.sync.drain()
    wait_clock.add_sem_waits(
        drain_inst.ins, _tile_mod.ScopedClock({None: tick_clock.global_clock})
    )
    si = drain_inst.ins.sync_info
    waits = list(si.on_wait) if si is not None else []
    if len(waits) > 1:
        drain_inst.ins.sync_info = _mybir.SyncInfo(on_update=[], on_wait=waits[:1])
        for w in waits[1:]:
            extra = nc.sync.nop(nofuse=True, hint="drain_wait_split")
            extra.ins.sync_info = _mybir.SyncInfo(on_update=[], on_wait=[w])
    nc.all_engine_barrier()
    assert self.sems is not None
    popped = nc._tile_sem_poison_stack.pop()
    assert popped is self._sem_poison
    nc.clear_and_free_semaphores(list(self.sems.allocated().values()))
    nc.all_engine_barrier()


_tile_mod.TileContext._drain_and_barrier = _patched_drain_and_barrier


def _split_multiwaits(nc):
    for fn in nc.m.functions:
        for blk in fn.blocks:
            out, changed = [], False
            for inst in list(blk.instructions):
                si = inst.sync_info
                waits = list(si.on_wait) if si is not None else []
                cap = 2 if inst.opcode == "EventSemaphore" else 1
                if len(waits) > cap:
                    changed = True
                    for idx, w in enumerate(waits[:-cap]):
                        nop = _mybir.InstNoOp(
                            name=f"{inst.name}-wsplit{idx}", ins=[], outs=[]
                        )
                        nop.engine = inst.engine
                        nop.sync_info = _mybir.SyncInfo(on_update=[], on_wait=[w])
                        out.append(nop)
                    inst.sync_info = _mybir.SyncInfo(
                        on_update=list(si.on_update), on_wait=waits[-cap:]
                    )
                out.append(inst)
            if changed:
                blk.instructions = out


# ---------------------------------------------------------------------------

F16 = mybir.dt.float16
F32 = mybir.dt.float32

N_CORES = 8
B, S, DIM, F = 32, 1024, 1024, 3072  # F = enc feature dim; DIM = model dim
KF = F // 128  # 24 enc k-tiles
KD = DIM // 128  # 8 hs k-tiles
EXP_SHIFT = -4.0  # w = exp(e + EXP_SHIFT); e is O(1), shift keeps fp16 safe


def _bcast_free(ap, n, at=1):
    """Insert a step-0 (broadcast) free dim of size n at position `at`."""
    aps = list(ap.ap)
    aps.insert(at, [0, n])
    return bass.AP(tensor=ap.tensor, offset=ap.offset, ap=aps)


def _bcast_part(ap_in, n=128):
    """Source AP that re-reads a single-partition row n times (for a DMA
    that replicates one SBUF row across n destination partitions)."""
    ap = ap_in[:] if not isinstance(ap_in, bass.AP) else ap_in
    aps = list(ap.ap)
    assert aps[0][1] == 1, "source must be single-partition"
    aps.insert(1, [0, n])
    return bass.AP(tensor=ap.tensor, offset=ap.offset, ap=aps)


def build_bass(nb, j_tiles):
    """nb batches per core, j_tiles row-tiles of 128 per batch."""
    nj = nb * j_tiles
    nc = bass.Bass()
    encT = nc.declare_dram_parameter("encT", [nj, 128, KF, 128], F16, isOutput=False)
    w1t = nc.declare_dram_parameter("w1t", [KF + KD, 128, DIM], F16, isOutput=False)
    w3t = nc.declare_dram_parameter("w3t", [KF, 128, DIM], F16, isOutput=False)
    hst = nc.declare_dram_parameter("hst", [KD, 128, nb], F16, isOutput=False)
    b1r = nc.declare_dram_parameter("b1r", [1, DIM], F16, isOutput=False)
    w2b = nc.declare_dram_parameter("w2b", [128, DIM], F16, isOutput=False)
    b3b = nc.declare_dram_parameter("b3b", [nb, DIM], F32, isOutput=False)
    onesb = nc.declare_dram_parameter("onesb", [128, 128], F16, isOutput=False)
    out_d = nc.declare_dram_parameter("out", [nb, DIM], F32, isOutput=True)

    with tile.TileContext(nc) as tc:
        with (
            tc.tile_pool(name="consts", bufs=1) as consts,
            tc.tile_pool(name="encT", bufs=4) as encT_pool,
            tc.tile_pool(name="tanh", bufs=2) as tanh_pool,
            tc.tile_pool(name="scratch", bufs=1) as scratch_pool,
            tc.tile_pool(name="prod", bufs=2) as prod_pool,
            tc.tile_pool(name="wrow", bufs=3) as wrow_pool,
            tc.tile_pool(name="ctxa", bufs=2) as ctxa_pool,
            tc.tile_pool(name="ps", bufs=3, space="PSUM") as ps,
        ):
            # ---- resident constants ----
            # DMA emission order is the schedule priority. The PE's first
            # dependency is et0 + w1t[0..], then the hb chain's hs chunks.
            w1t_sb = consts.tile([128, KF + KD, DIM], F16)
            et0 = encT_pool.tile([128, KF, 128], F16, tag="et")
            for q in range(4):
                nc.sync.dma_start(
                    out=et0[:, 6 * q : 6 * (q + 1), :],
                    in_=encT[0][:, 6 * q : 6 * (q + 1), :],
                )
            for k in range(4):
                nc.sync.dma_start(out=w1t_sb[:, k, :], in_=w1t[k])
            hst_sb = consts.tile([128, KD, nb], F16)
            for k in range(KD):
                nc.sync.dma_start(out=hst_sb[:, k, :], in_=hst[k])
            b1_sb = consts.tile([1, DIM], F16)
            nc.sync.dma_start(out=b1_sb, in_=b1r[:])
            w2b_sb = consts.tile([128, DIM], F16)
            nc.sync.dma_start(out=w2b_sb, in_=w2b[:])
            ones_sb = consts.tile([128, 128], F16)
            nc.sync.dma_start(out=ones_sb, in_=onesb[:])
            for k in range(KD):  # hs chunks: the hb stage needs them
                nc.sync.dma_start(out=w1t_sb[:, KF + k, :], in_=w1t[KF + k])
            for k in range(4, KF):
                nc.sync.dma_start(out=w1t_sb[:, k, :], in_=w1t[k])
            # tail-only constants declared here, loaded late (low priority)
            w3t_sb = consts.tile([128, KF, DIM], F16)
            b3_sb = consts.tile([nb, DIM], F32)

            negc_sb = consts.tile([128, 1], F32)
            nc.vector.memset(negc_sb, EXP_SHIFT)

            hb_sb = consts.tile([nb, DIM], F16)
            hbflat_sb = consts.tile([1, nb, DIM], F16)
            e_sb = consts.tile([128, nj], F32)
            lparts_sb = consts.tile([1, nb, j_tiles], F32)
            linv_sb = consts.tile([1, nb], F32)
            invl_sb = consts.tile([nb, 1], F32)
            ctxT_sb = consts.tile([128, KF, nb], F16)
            out_sb = consts.tile([nb, DIM], F32)

            def emit_hb_stage():
                # hb = hs @ W1h.T + b1 (per-batch bias rows). Own PSUM tag so
                # it can sit between row-tile 0's k-loops without stealing
                # their slots. Emitted after row-tile 0's main matmuls so the
                # PE doesn't idle waiting for the (later-loaded) hs chunks.
                for nh in range(2):
                    sl = ds(nh * 512, 512)
                    hbp = ps.tile([nb, 512], F32, tag="hb", bufs=1)
                    for k in range(KD):
                        nc.tensor.matmul(
                            hbp,
                            hst_sb[:, k, :],
                            w1t_sb[:, KF + k, sl],
                            start=(k == 0),
                            stop=False,
                        )
                    nc.tensor.matmul(
                        hbp,
                        ones_sb[0:1, 0:nb],
                        b1_sb[0:1, sl],
                        start=False,
                        stop=True,
                    )
                    nc.vector.tensor_copy(hb_sb[:, sl], hbp)
                # gather the per-batch bias rows onto partition 0 (matmul rhs
                # operands must start at partition 0)
                nc.sync.dma_start(out=hbflat_sb, in_=hb_sb)

            # ---- main loop ----
            # The ctx chain for row-tile j (w broadcast + DVE mul/reduce) is
            # emitted after row-tile j+1's matmuls so the PE never waits on
            # the tanh->e->exp->spread chain.
            ctx_accs = {}
            pending = None

            def _emit_bias_tanh(b, nh, hp, th):
                sl = ds(nh * 512, 512)
                # bias last so the group doesn't wait on the hb chain
                nc.tensor.matmul(
                    hp,
                    ones_sb[0:1, :],
                    hbflat_sb[0:1, b, sl],
                    start=False,
                    stop=True,
                )
                nc.scalar.activation(
                    th[:, sl], hp, mybir.ActivationFunctionType.Tanh
                )

            def emit_ctx_tail(state):
                b, j, et, wr = state
                ctx_acc = ctx_accs[b]
                # broadcast w across partitions via K=1 outer product
                wbp = ps.tile([128, 128], F32, tag="wb", bufs=2)
                nc.tensor.matmul(wbp, ones_sb[0:1, :], wr, start=True, stop=True)
                wb = wrow_pool.tile([128, 128], F16, tag="wb")
                nc.vector.tensor_copy(wb, wbp)
                # ctx_partial[f-chunk c] = sum_s wb[:, s] * et[:, c, s]
                pr = prod_pool.tile([128, KF, 128], F16)
                nc.vector.tensor_mul(pr, et, _bcast_free(wb[:], KF))
                cpart = ctxa_pool.tile([128, KF], F32, tag="cpart")
                nc.vector.tensor_reduce(
                    out=cpart,
                    in_=pr,
                    axis=mybir.AxisListType.X,
                    op=mybir.AluOpType.add,
                )
                if j == 0:
                    nc.vector.tensor_copy(ctx_acc, cpart)
                else:
                    nc.vector.tensor_add(ctx_acc, ctx_acc, cpart)
                if j == j_tiles - 1:
                    # ctxT column for this batch (f16 for the W3 matmuls)
                    nc.vector.tensor_copy(ctxT_sb[:, :, b], ctx_acc)

            for b in range(nb):
                ctx_acc_b = ctxa_pool.tile([128, KF], F32, tag="ctx_acc")
                ctx_accs[b] = ctx_acc_b
                for j in range(j_tiles):
                    jj = b * j_tiles + j
                    if jj == 0:
                        et = et0
                    else:
                        et = encT_pool.tile([128, KF, 128], F16, tag="et")
                        nc.sync.dma_start(out=et, in_=encT[jj])
                    if 4 <= jj < 4 + KF:
                        nc.sync.dma_start(
                            out=w3t_sb[:, jj - 4, :], in_=w3t[jj - 4]
                        )
                    elif jj == 4 + KF:
                        nc.sync.dma_start(out=b3_sb, in_=b3b[:])
                    th = tanh_pool.tile([128, DIM], F16)
                    hps = []
                    for nh in range(2):
                        sl = ds(nh * 512, 512)
                        hp = ps.tile([128, 512], F32, tag="h")
                        hps.append(hp)
                        for k in range(KF):
                            nc.tensor.matmul(
                                hp,
                                et[:, k, :],
                                w1t_sb[:, k, sl],
                                start=(k == 0),
                                stop=False,
                            )
                        if jj > 0:
                            _emit_bias_tanh(b, nh, hp, th)
                    if jj == 0:
                        # row-tile 0: hb stage runs after the main k-loops so
                        # the PE starts on enc data, not on the hb chain
                        emit_hb_stage()
                        for nh in range(2):
                            _emit_bias_tanh(b, nh, hps[nh], th)
                    sc = scratch_pool.tile([128, DIM], F16)
                    nc.vector.scalar_tensor_tensor(
                        out=sc,
                        in0=th,
                        scalar=1.0,
                        in1=w2b_sb,
                        op0=mybir.AluOpType.mult,
                        op1=mybir.AluOpType.mult,
                        accum_out=e_sb[:, jj : jj + 1],
                    )
                    # w = exp(e-4) as a column, spread to a row via DMA
                    wc = wrow_pool.tile([128, 1], F16, tag="wc")
                    nc.scalar.activation(
                        wc,
                        e_sb[:, jj : jj + 1],
                        mybir.ActivationFunctionType.Exp,
                        bias=negc_sb,
                    )
                    wr = wrow_pool.tile([1, 128], F16)
                    nc.sync.dma_start(out=wr, in_=wc)
                    nc.vector.tensor_reduce(
                        out=lparts_sb[0:1, b, j : j + 1],
                        in_=wr,
                        axis=mybir.AxisListType.X,
                        op=mybir.AluOpType.add,
                    )
                    if pending is not None:
                        emit_ctx_tail(pending)
                    pending = (b, j, et, wr)
            emit_ctx_tail(pending)

            # ---- 1/l per batch, spread to a partition-column ----
            nc.vector.tensor_reduce(
                out=linv_sb,
                in_=lparts_sb,
                axis=mybir.AxisListType.X,
                op=mybir.AluOpType.add,
            )
            nc.vector.reciprocal(linv_sb, linv_sb)
            nc.sync.dma_start(out=invl_sb, in_=linv_sb[0:1, :])

            # ---- out = (ctx @ W3.T) * inv_l + b3 ----
            for nh in range(2):
                sl = ds(nh * 512, 512)
                wp = ps.tile([nb, 512], F32, tag="h")
                for k in range(KF):
                    nc.tensor.matmul(
                        wp,
                        ctxT_sb[:, k, :],
                        w3t_sb[:, k, sl],
                        start=(k == 0),
                        stop=(k == KF - 1),
                    )
                nc.vector.scalar_tensor_tensor(
                    out=out_sb[:, sl],
                    in0=wp,
                    scalar=invl_sb,
                    in1=b3_sb[:, sl],
                    op0=mybir.AluOpType.mult,
                    op1=mybir.AluOpType.add,
                )
            nc.sync.dma_start(out=out_d[:], in_=out_sb)

    _split_multiwaits(nc)
    return nc


def make_in_maps(hidden_state, encoder_outputs, W1, b1, w2, W3, b3, nb, j_tiles):
    """Shard + lay out the full inputs for each core. Returns list of dicts."""
    f16, f32 = np.float16, np.float32
    nj = nb * j_tiles
    s_core = j_tiles * 128

    w1t = np.ascontiguousarray(W1.T.reshape(KF + KD, 128, DIM)).astype(f16)
    w3t = np.ascontiguousarray(W3.T.reshape(KF, 128, DIM)).astype(f16)
    b1r = b1.reshape(1, DIM).astype(f16)
    w2b = np.ascontiguousarray(np.broadcast_to(w2.reshape(1, DIM), (128, DIM))).astype(
        f16
    )
    onesb = np.ones((128, 128), f16)
    b3b_full = np.ascontiguousarray(
        np.broadcast_to(b3.reshape(1, DIM), (nb, DIM))
    ).astype(f32)

    in_maps = []
    for i in range(N_CORES):
        bs = slice(i * nb, (i + 1) * nb)
        enc_c = encoder_outputs[bs, :s_core, :]  # (nb, s_core, F)
        e5 = enc_c.reshape(nb, j_tiles, 128, KF, 128)
        encT = np.ascontiguousarray(e5.transpose(0, 1, 4, 3, 2)).astype(f16)
        hs_c = hidden_state[bs]  # (nb, DIM)
        hst = np.ascontiguousarray(hs_c.T.reshape(KD, 128, nb)).astype(f16)
        in_maps.append(
            {
                "encT": encT.reshape(nj, 128, KF, 128),
                "w1t": w1t,
                "w3t": w3t,
                "hst": hst,
                "b1r": b1r,
                "w2b": w2b,
                "b3b": b3b_full,
                "onesb": onesb,
            }
        )
    return in_maps


_CACHE = {}


def run(hidden_state, encoder_outputs, W1, b1, w2, W3, b3, nb, j_tiles, trace=False):
    key = (nb, j_tiles)
    if key not in _CACHE:
        _CACHE[key] = build_bass(nb, j_tiles)
    nc = _CACHE[key]
    in_maps = make_in_maps(
        hidden_state, encoder_outputs, W1, b1, w2, W3, b3, nb, j_tiles
    )
    res = bass_utils.run_bass_kernel_spmd(
        nc, in_maps, list(range(N_CORES)), trace=trace
    )
    out = np.concatenate([res.results[i]["out"] for i in range(N_CORES)], axis=0)
    return out.astype(np.float32), res


def kernel(hidden_state, encoder_outputs, W1, b1, w2, W3, b3):
    hidden_state = np.asarray(hidden_state, dtype=np.float32)
    encoder_outputs = np.asarray(encoder_outputs, dtype=np.float32)
    W1 = np.asarray(W1, dtype=np.float32)
    b1 = np.asarray(b1, dtype=np.float32)
    w2 = np.asarray(w2, dtype=np.float32)
    W3 = np.asarray(W3, dtype=np.float32)
    b3 = np.asarray(b3, dtype=np.float32)
    out, _ = run(hidden_state, encoder_outputs, W1, b1, w2, W3, b3, nb=4, j_tiles=8)
    return out


# revision 28
# speedup vs baseline: 1.0041x; 1.0041x over previous
"""Trainium2 Bass kernel for the aux-attention module.

reference (per batch b):
    inputs = concat([enc[b], broadcast(hs[b])], -1)          # (S, 4096)
    hidden = tanh(inputs @ W1.T + b1)                        # (S, 1024)
    e      = hidden @ w2.T                                   # (S,)
    alpha  = softmax(e)
    ctx    = alpha @ enc[b]                                  # (3072,)
    out[b] = ctx @ W3.T + b3                                 # (1024,)

Strategy: data-parallel over batch (4 batches/core x 8 cores), weights
replicated. All PE matmuls in fp16 (fp32 PSUM accumulation). Softmax without
max-subtraction: w = exp(e - 4) unnormalized (e is O(1) for this model), the
1/sum(w) normalization is folded into the final output scaling.

Per core, per 128-row tile (single pass over enc, f-major layout from host):
  - hidden = tanh(enc_tile @ W1e.T + hb) on PE (25 N=512 matmuls) + ACT
  - e column via one fused DVE multiply+accumulate against broadcast w2
  - e -> row (PE transpose), w = exp(e-4) (ACT, also accumulates l), w
    broadcast across partitions (K=1 matmul), then ctx_partial[f-chunk] =
    sum_s w[s]*enc[f, s] as a DVE multiply + per-chunk reduce on the same
    f-major tile already in SBUF (no second HBM read of enc).
Tail: inv_l via reduce+reciprocal, out = (ctxT @ W3.T) * inv_l + b3.
"""

import numpy as np

import concourse.bass as bass
import concourse.tile as tile
from concourse import mybir
from concourse.bass import ds
from concourse import bass_utils

# ---------------------------------------------------------------------------
# Walrus in this container caps sync waits per instruction (one; two for
# EventSemaphore). Tile's tail drain carries one wait per live semaphore and
# Tile occasionally leaks multi-wait instructions; split extras onto cheap
# carriers.
from concourse import tile as _tile_mod
from concourse import mybir as _mybir


def _patched_drain_and_barrier(self, tick_clock, wait_clock):
    nc = self.nc
    drain_inst = nc.sync.drain()
    wait_clock.add_sem_waits(
        drain_inst.ins, _tile_mod.ScopedClock({None: tick_clock.global_clock})
    )
    si = drain_inst.ins.sync_info
    waits = list(si.on_wait) if si is not None else []
    if len(waits) > 1:
        drain_inst.ins.sync_info = _mybir.SyncInfo(on_update=[], on_wait=waits[:1])
        for w in waits[1:]:
            extra = nc.sync.nop(nofuse=True, hint="drain_wait_split")
            extra.ins.sync_info = _mybir.SyncInfo(on_update=[], on_wait=[w])
    nc.all_engine_barrier()
    assert self.sems is not None
    popped = nc._tile_sem_poison_stack.pop()
    assert popped is self._sem_poison
    nc.clear_and_free_semaphores(list(self.sems.allocated().values()))
    nc.all_engine_barrier()


_tile_mod.TileContext._drain_and_barrier = _patched_drain_and_barrier


def _split_multiwaits(nc):
    for fn in nc.m.functions:
        for blk in fn.blocks:
            out, changed = [], False
            for inst in list(blk.instructions):
                si = inst.sync_info
                waits = list(si.on_wait) if si is not None else []
                cap = 2 if inst.opcode == "EventSemaphore" else 1
                if len(waits) > cap:
                    changed = True
                    for idx, w in enumerate(waits[:-cap]):
                        nop = _mybir.InstNoOp(
                            name=f"{inst.name}-wsplit{idx}", ins=[], outs=[]
                        )
                        nop.engine = inst.engine
                        nop.sync_info = _mybir.SyncInfo(on_update=[], on_wait=[w])
                        out.append(nop)
                    inst.sync_info = _mybir.SyncInfo(
                        on_update=list(si.on_update), on_wait=waits[-cap:]
                    )
                out.append(inst)
            if changed:
                blk.instructions = out


# ---------------------------------------------------------------------------

F16 = mybir.dt.float16
F32 = mybir.dt.float32

N_CORES = 8
B, S, DIM, F = 32, 1024, 1024, 3072  # F = enc feature dim; DIM = model dim
KF = F // 128  # 24 enc k-tiles
KD = DIM // 128  # 8 hs k-tiles
EXP_SHIFT = -4.0  # w = exp(e + EXP_SHIFT); e is O(1), shift keeps fp16 safe


def _bcast_free(ap, n, at=1):
    """Insert a step-0 (broadcast) free dim of size n at position `at`."""
    aps = list(ap.ap)
    aps.insert(at, [0, n])
    return bass.AP(tensor=ap.tensor, offset=ap.offset, ap=aps)


def _bcast_part(ap_in, n=128):
    """Source AP that re-reads a single-partition row n times (for a DMA
    that replicates one SBUF row across n destination partitions)."""
    ap = ap_in[:] if not isinstance(ap_in, bass.AP) else ap_in
    aps = list(ap.ap)
    assert aps[0][1] == 1, "source must be single-partition"
    aps.insert(1, [0, n])
    return bass.AP(tensor=ap.tensor, offset=ap.offset, ap=aps)


def build_bass(nb, j_tiles):
    """nb batches per core, j_tiles row-tiles of 128 per batch."""
    nj = nb * j_tiles
    nc = bass.Bass()
    encT = nc.declare_dram_parameter("encT", [nj, 128, KF, 128], F16, isOutput=False)
    w1t = nc.declare_dram_parameter("w1t", [KF + KD, 128, DIM], F16, isOutput=False)
    w3t = nc.declare_dram_parameter("w3t", [KF, 128, DIM], F16, isOutput=False)
    hst = nc.declare_dram_parameter("hst", [KD, 128, nb], F16, isOutput=False)
    b1r = nc.declare_dram_parameter("b1r", [1, DIM], F16, isOutput=False)
    w2b = nc.declare_dram_parameter("w2b", [128, DIM], F16, isOutput=False)
    b3b = nc.declare_dram_parameter("b3b", [nb, DIM], F32, isOutput=False)
    onesb = nc.declare_dram_parameter("onesb", [128, 128], F16, isOutput=False)
    out_d = nc.declare_dram_parameter("out", [nb, DIM], F32, isOutput=True)

    with tile.TileContext(nc) as tc:
        with (
            tc.tile_pool(name="consts", bufs=1) as consts,
            tc.tile_pool(name="encT", bufs=4) as encT_pool,
            tc.tile_pool(name="tanh", bufs=2) as tanh_pool,
            tc.tile_pool(name="scratch", bufs=1) as scratch_pool,
            tc.tile_pool(name="prod", bufs=2) as prod_pool,
            tc.tile_pool(name="wrow", bufs=3) as wrow_pool,
            tc.tile_pool(name="ctxa", bufs=2) as ctxa_pool,
            tc.tile_pool(name="ps", bufs=3, space="PSUM") as ps,
        ):
            # ---- resident constants ----
            # DMA emission order is the schedule priority. The PE's first
            # dependency is et0 + w1t[0..], then the hb chain's hs chunks.
            w1t_sb = consts.tile([128, KF + KD, DIM], F16)
            et0 = encT_pool.tile([128, KF, 128], F16, tag="et")
            for q in range(4):
                nc.sync.dma_start(
                    out=et0[:, 6 * q : 6 * (q + 1), :],
                    in_=encT[0][:, 6 * q : 6 * (q + 1), :],
                )
            for k in range(4):
                nc.sync.dma_start(out=w1t_sb[:, k, :], in_=w1t[k])
            hst_sb = consts.tile([128, KD, nb], F16)
            for k in range(KD):
                nc.sync.dma_start(out=hst_sb[:, k, :], in_=hst[k])
            b1_sb = consts.tile([1, DIM], F16)
            nc.sync.dma_start(out=b1_sb, in_=b1r[:])
            w2b_sb = consts.tile([128, DIM], F16)
            nc.sync.dma_start(out=w2b_sb, in_=w2b[:])
            ones_sb = consts.tile([128, 128], F16)
            nc.sync.dma_start(out=ones_sb, in_=onesb[:])
            for k in range(KD):  # hs chunks: the hb stage needs them
                nc.sync.dma_start(out=w1t_sb[:, KF + k, :], in_=w1t[KF + k])
            for k in range(4, KF):
                nc.sync.dma_start(out=w1t_sb[:, k, :], in_=w1t[k])
            # tail-only constants declared here, loaded late (low priority)
            w3t_sb = consts.tile([128, KF, DIM], F16)
            b3_sb = consts.tile([nb, DIM], F32)

            negc_sb = consts.tile([128, 1], F32)
            nc.vector.memset(negc_sb, EXP_SHIFT)

            hb_sb = consts.tile([nb, DIM], F16)
            hbflat_sb = consts.tile([1, nb, DIM], F16)
            e_sb = consts.tile([128, nj], F32)
            lparts_sb = consts.tile([1, nb, j_tiles], F32)
            linv_sb = consts.tile([1, nb], F32)
            invl_sb = consts.tile([nb, 1], F32)
            ctxT_sb = consts.tile([128, KF, nb], F16)
            out_sb = consts.tile([nb, DIM], F32)

            def emit_hb_stage():
                # hb = hs @ W1h.T + b1 (per-batch bias rows). Own PSUM tag so
                # it can sit between row-tile 0's k-loops without stealing
                # their slots. Emitted after row-tile 0's main matmuls so the
                # PE doesn't idle waiting for the (later-loaded) hs chunks.
                for nh in range(2):
                    sl = ds(nh * 512, 512)
                    hbp = ps.tile([nb, 512], F32, tag="hb", bufs=1)
                    for k in range(KD):
                        nc.tensor.matmul(
                            hbp,
                            hst_sb[:, k, :],
                            w1t_sb[:, KF + k, sl],
                            start=(k == 0),
                            stop=False,
                        )
                    nc.tensor.matmul(
                        hbp,
                        ones_sb[0:1, 0:nb],
                        b1_sb[0:1, sl],
                        start=False,
                        stop=True,
                    )
                    nc.vector.tensor_copy(hb_sb[:, sl], hbp)
                # gather the per-batch bias rows onto partition 0 (matmul rhs
                # operands must start at partition 0)
                nc.sync.dma_start(out=hbflat_sb, in_=hb_sb)

            # ---- main loop ----
            # The ctx chain for row-tile j (w broadcast + DVE mul/reduce) is
            # emitted after row-tile j+1's matmuls so the PE never waits on
            # the tanh->e->exp->spread chain.
            ctx_accs = {}
            pending = None

            def _emit_bias_tanh(b, nh, hp, th):
                sl = ds(nh * 512, 512)
                # bias last so the group doesn't wait on the hb chain
                nc.tensor.matmul(
                    hp,
                    ones_sb[0:1, :],
                    hbflat_sb[0:1, b, sl],
                    start=False,
                    stop=True,
                )
                nc.scalar.activation(
                    th[:, sl], hp, mybir.ActivationFunctionType.Tanh
                )

            def emit_ctx_tail(state):
                b, j, et, wr = state
                ctx_acc = ctx_accs[b]
                # broadcast w across partitions via K=1 outer product
                wbp = ps.tile([128, 128], F32, tag="wb", bufs=2)
                nc.tensor.matmul(wbp, ones_sb[0:1, :], wr, start=True, stop=True)
                wb = wrow_pool.tile([128, 128], F16, tag="wb")
                nc.vector.tensor_copy(wb, wbp)
                # ctx_partial[f-chunk c] = sum_s wb[:, s] * et[:, c, s]
                pr = prod_pool.tile([128, KF, 128], F16)
                nc.vector.tensor_mul(pr, et, _bcast_free(wb[:], KF))
                cpart = ctxa_pool.tile([128, KF], F32, tag="cpart")
                nc.vector.tensor_reduce(
                    out=cpart,
                    in_=pr,
                    axis=mybir.AxisListType.X,
                    op=mybir.AluOpType.add,
                )
                if j == 0:
                    nc.vector.tensor_copy(ctx_acc, cpart)
                else:
                    nc.vector.tensor_add(ctx_acc, ctx_acc, cpart)
                if j == j_tiles - 1:
                    # ctxT column for this batch (f16 for the W3 matmuls)
                    nc.vector.tensor_copy(ctxT_sb[:, :, b], ctx_acc)

            for b in range(nb):
                ctx_acc_b = ctxa_pool.tile([128, KF], F32, tag="ctx_acc")
                ctx_accs[b] = ctx_acc_b
                for j in range(j_tiles):
                    jj = b * j_tiles + j
                    if jj == 0:
                        et = et0
                    else:
                        et = encT_pool.tile([128, KF, 128], F16, tag="et")
                        nc.sync.dma_start(out=et, in_=encT[jj])
                    # spread the w3t prefetch across the main loop
                    if jj >= min(4, nj - 1):
                        span = max(nj - min(4, nj - 1), 1)
                        pos = jj - min(4, nj - 1)
                        lo, hi = pos * KF // span, (pos + 1) * KF // span
                        for kk in range(lo, min(hi, KF)):
                            nc.sync.dma_start(out=w3t_sb[:, kk, :], in_=w3t[kk])
                    if jj == nj - 1:
                        nc.sync.dma_start(out=b3_sb, in_=b3b[:])
                    th = tanh_pool.tile([128, DIM], F16)
                    hps = []
                    for nh in range(2):
                        sl = ds(nh * 512, 512)
                        hp = ps.tile([128, 512], F32, tag="h")
                        hps.append(hp)
                        for k in range(KF):
                            nc.tensor.matmul(
                                hp,
                                et[:, k, :],
                                w1t_sb[:, k, sl],
                                start=(k == 0),
                                stop=False,
                            )
                        if jj > 0:
                            _emit_bias_tanh(b, nh, hp, th)
                    if jj == 0:
                        # row-tile 0: hb stage runs after the main k-loops so
                        # the PE starts on enc data, not on the hb chain
                        emit_hb_stage()
                        for nh in range(2):
                            _emit_bias_tanh(b, nh, hps[nh], th)
                    sc = scratch_pool.tile([128, DIM], F16)
                    nc.vector.scalar_tensor_tensor(
                        out=sc,
                        in0=th,
                        scalar=1.0,
                        in1=w2b_sb,
                        op0=mybir.AluOpType.mult,
                        op1=mybir.AluOpType.mult,
                        accum_out=e_sb[:, jj : jj + 1],
                    )
                    # w = exp(e-4) as a column, spread to a row via DMA
                    wc = wrow_pool.tile([128, 1], F16, tag="wc")
                    nc.scalar.activation(
                        wc,
                        e_sb[:, jj : jj + 1],
                        mybir.ActivationFunctionType.Exp,
                        bias=negc_sb,
                    )
                    wr = wrow_pool.tile([1, 128], F16)
                    nc.sync.dma_start(out=wr, in_=wc)
                    nc.vector.tensor_reduce(
                        out=lparts_sb[0:1, b, j : j + 1],
                        in_=wr,
                        axis=mybir.AxisListType.X,
                        op=mybir.AluOpType.add,
                    )
                    if pending is not None:
                        emit_ctx_tail(pending)
                    pending = (b, j, et, wr)
            emit_ctx_tail(pending)

            # ---- 1/l per batch, spread to a partition-column ----
            nc.vector.tensor_reduce(
                out=linv_sb,
                in_=lparts_sb,
                axis=mybir.AxisListType.X,
                op=mybir.AluOpType.add,
            )
            nc.vector.reciprocal(linv_sb, linv_sb)
            nc.sync.dma_start(out=invl_sb, in_=linv_sb[0:1, :])

            # ---- out = (ctx @ W3.T) * inv_l + b3 ----
            for nh in range(2):
                sl = ds(nh * 512, 512)
                wp = ps.tile([nb, 512], F32, tag="h")
                for k in range(KF):
                    nc.tensor.matmul(
                        wp,
                        ctxT_sb[:, k, :],
                        w3t_sb[:, k, sl],
                        start=(k == 0),
                        stop=(k == KF - 1),
                    )
                nc.vector.scalar_tensor_tensor(
                    out=out_sb[:, sl],
                    in0=wp,
                    scalar=invl_sb,
                    in1=b3_sb[:, sl],
                    op0=mybir.AluOpType.mult,
                    op1=mybir.AluOpType.add,
                )
            nc.sync.dma_start(out=out_d[:], in_=out_sb)

    _split_multiwaits(nc)
    return nc


def make_in_maps(hidden_state, encoder_outputs, W1, b1, w2, W3, b3, nb, j_tiles):
    """Shard + lay out the full inputs for each core. Returns list of dicts."""
    f16, f32 = np.float16, np.float32
    nj = nb * j_tiles
    s_core = j_tiles * 128

    w1t = np.ascontiguousarray(W1.T.reshape(KF + KD, 128, DIM)).astype(f16)
    w3t = np.ascontiguousarray(W3.T.reshape(KF, 128, DIM)).astype(f16)
    b1r = b1.reshape(1, DIM).astype(f16)
    w2b = np.ascontiguousarray(np.broadcast_to(w2.reshape(1, DIM), (128, DIM))).astype(
        f16
    )
    onesb = np.ones((128, 128), f16)
    b3b_full = np.ascontiguousarray(
        np.broadcast_to(b3.reshape(1, DIM), (nb, DIM))
    ).astype(f32)

    in_maps = []
    for i in range(N_CORES):
        bs = slice(i * nb, (i + 1) * nb)
        enc_c = encoder_outputs[bs, :s_core, :]  # (nb, s_core, F)
        e5 = enc_c.reshape(nb, j_tiles, 128, KF, 128)
        encT = np.ascontiguousarray(e5.transpose(0, 1, 4, 3, 2)).astype(f16)
        hs_c = hidden_state[bs]  # (nb, DIM)
        hst = np.ascontiguousarray(hs_c.T.reshape(KD, 128, nb)).astype(f16)
        in_maps.append(
            {
                "encT": encT.reshape(nj, 128, KF, 128),
                "w1t": w1t,
                "w3t": w3t,
                "hst": hst,
                "b1r": b1r,
                "w2b": w2b,
                "b3b": b3b_full,
                "onesb": onesb,
            }
        )
    return in_maps


_CACHE = {}


def run(hidden_state, encoder_outputs, W1, b1, w2, W3, b3, nb, j_tiles, trace=False):
    key = (nb, j_tiles)
    if key not in _CACHE:
        _CACHE[key] = build_bass(nb, j_tiles)
    nc = _CACHE[key]
    in_maps = make_in_maps(
        hidden_state, encoder_outputs, W1, b1, w2, W3, b3, nb, j_tiles
    )
    res = bass_utils.run_bass_kernel_spmd(
        nc, in_maps, list(range(N_CORES)), trace=trace
    )
    out = np.concatenate([res.results[i]["out"] for i in range(N_CORES)], axis=0)
    return out.astype(np.float32), res


def kernel(hidden_state, encoder_outputs, W1, b1, w2, W3, b3):
    hidden_state = np.asarray(hidden_state, dtype=np.float32)
    encoder_outputs = np.asarray(encoder_outputs, dtype=np.float32)
    W1 = np.asarray(W1, dtype=np.float32)
    b1 = np.asarray(b1, dtype=np.float32)
    w2 = np.asarray(w2, dtype=np.float32)
    W3 = np.asarray(W3, dtype=np.float32)
    b3 = np.asarray(b3, dtype=np.float32)
    out, _ = run(hidden_state, encoder_outputs, W1, b1, w2, W3, b3, nb=4, j_tiles=8)
    return out


# revision 29
# speedup vs baseline: 1.0125x; 1.0084x over previous
"""Trainium2 Bass kernel for the aux-attention module.

reference (per batch b):
    inputs = concat([enc[b], broadcast(hs[b])], -1)          # (S, 4096)
    hidden = tanh(inputs @ W1.T + b1)                        # (S, 1024)
    e      = hidden @ w2.T                                   # (S,)
    alpha  = softmax(e)
    ctx    = alpha @ enc[b]                                  # (3072,)
    out[b] = ctx @ W3.T + b3                                 # (1024,)

Strategy: data-parallel over batch (4 batches/core x 8 cores), weights
replicated. All PE matmuls in fp16 (fp32 PSUM accumulation). Softmax without
max-subtraction: w = exp(e - 4) unnormalized (e is O(1) for this model), the
1/sum(w) normalization is folded into the final output scaling.

Per core, per 128-row tile (single pass over enc, f-major layout from host):
  - hidden = tanh(enc_tile @ W1e.T + hb) on PE (25 N=512 matmuls) + ACT
  - e column via one fused DVE multiply+accumulate against broadcast w2
  - e -> row (PE transpose), w = exp(e-4) (ACT, also accumulates l), w
    broadcast across partitions (K=1 matmul), then ctx_partial[f-chunk] =
    sum_s w[s]*enc[f, s] as a DVE multiply + per-chunk reduce on the same
    f-major tile already in SBUF (no second HBM read of enc).
Tail: inv_l via reduce+reciprocal, out = (ctxT @ W3.T) * inv_l + b3.
"""

import numpy as np

try:  # persistent compile cache: repeated runs skip the walrus compile
    import jax

    jax.config.update("jax_compilation_cache_dir", "/tmp/jax_neff_cache")
    jax.config.update("jax_persistent_cache_min_compile_time_secs", 1.0)
except Exception:
    pass

import concourse.bass as bass
import concourse.tile as tile
from concourse import mybir
from concourse.bass import ds
from concourse import bass_utils

# ---------------------------------------------------------------------------
# Walrus in this container caps sync waits per instruction (one; two for
# EventSemaphore). Tile's tail drain carries one wait per live semaphore and
# Tile occasionally leaks multi-wait instructions; split extras onto cheap
# carriers.
from concourse import tile as _tile_mod
from concourse import mybir as _mybir


def _patched_drain_and_barrier(self, tick_clock, wait_clock):
    nc = self.nc
    drain_inst = nc.sync.drain()
    wait_clock.add_sem_waits(
        drain_inst.ins, _tile_mod.ScopedClock({None: tick_clock.global_clock})
    )
    si = drain_inst.ins.sync_info
    waits = list(si.on_wait) if si is not None else []
    if len(waits) > 1:
        drain_inst.ins.sync_info = _mybir.SyncInfo(on_update=[], on_wait=waits[:1])
        for w in waits[1:]:
            extra = nc.sync.nop(nofuse=True, hint="drain_wait_split")
            extra.ins.sync_info = _mybir.SyncInfo(on_update=[], on_wait=[w])
    nc.all_engine_barrier()
    assert self.sems is not None
    popped = nc._tile_sem_poison_stack.pop()
    assert popped is self._sem_poison
    nc.clear_and_free_semaphores(list(self.sems.allocated().values()))
    nc.all_engine_barrier()


_tile_mod.TileContext._drain_and_barrier = _patched_drain_and_barrier


def _split_multiwaits(nc):
    for fn in nc.m.functions:
        for blk in fn.blocks:
            out, changed = [], False
            for inst in list(blk.instructions):
                si = inst.sync_info
                waits = list(si.on_wait) if si is not None else []
                cap = 2 if inst.opcode == "EventSemaphore" else 1
                if len(waits) > cap:
                    changed = True
                    for idx, w in enumerate(waits[:-cap]):
                        nop = _mybir.InstNoOp(
                            name=f"{inst.name}-wsplit{idx}", ins=[], outs=[]
                        )
                        nop.engine = inst.engine
                        nop.sync_info = _mybir.SyncInfo(on_update=[], on_wait=[w])
                        out.append(nop)
                    inst.sync_info = _mybir.SyncInfo(
                        on_update=list(si.on_update), on_wait=waits[-cap:]
                    )
                out.append(inst)
            if changed:
                blk.instructions = out


# ---------------------------------------------------------------------------

F16 = mybir.dt.float16
F32 = mybir.dt.float32

N_CORES = 8
B, S, DIM, F = 32, 1024, 1024, 3072  # F = enc feature dim; DIM = model dim
KF = F // 128  # 24 enc k-tiles
KD = DIM // 128  # 8 hs k-tiles
EXP_SHIFT = -4.0  # w = exp(e + EXP_SHIFT); e is O(1), shift keeps fp16 safe


def _bcast_free(ap, n, at=1):
    """Insert a step-0 (broadcast) free dim of size n at position `at`."""
    aps = list(ap.ap)
    aps.insert(at, [0, n])
    return bass.AP(tensor=ap.tensor, offset=ap.offset, ap=aps)


def _bcast_part(ap_in, n=128):
    """Source AP that re-reads a single-partition row n times (for a DMA
    that replicates one SBUF row across n destination partitions)."""
    ap = ap_in[:] if not isinstance(ap_in, bass.AP) else ap_in
    aps = list(ap.ap)
    assert aps[0][1] == 1, "source must be single-partition"
    aps.insert(1, [0, n])
    return bass.AP(tensor=ap.tensor, offset=ap.offset, ap=aps)


def build_bass(nb, j_tiles):
    """nb batches per core, j_tiles row-tiles of 128 per batch."""
    nj = nb * j_tiles
    nc = bass.Bass()
    encT = nc.declare_dram_parameter("encT", [nj, 128, KF, 128], F16, isOutput=False)
    w1t = nc.declare_dram_parameter("w1t", [KF + KD, 128, DIM], F16, isOutput=False)
    w3t = nc.declare_dram_parameter("w3t", [KF, 128, DIM], F16, isOutput=False)
    hst = nc.declare_dram_parameter("hst", [KD, 128, nb], F16, isOutput=False)
    b1r = nc.declare_dram_parameter("b1r", [1, DIM], F16, isOutput=False)
    w2b = nc.declare_dram_parameter("w2b", [128, DIM], F16, isOutput=False)
    b3b = nc.declare_dram_parameter("b3b", [nb, DIM], F32, isOutput=False)
    onesb = nc.declare_dram_parameter("onesb", [128, 128], F16, isOutput=False)
    out_d = nc.declare_dram_parameter("out", [nb, DIM], F32, isOutput=True)

    with tile.TileContext(nc) as tc:
        with (
            tc.tile_pool(name="consts", bufs=1) as consts,
            tc.tile_pool(name="encT", bufs=4) as encT_pool,
            tc.tile_pool(name="tanh", bufs=2) as tanh_pool,
            tc.tile_pool(name="scratch", bufs=1) as scratch_pool,
            tc.tile_pool(name="prod", bufs=2) as prod_pool,
            tc.tile_pool(name="wrow", bufs=3) as wrow_pool,
            tc.tile_pool(name="ctxa", bufs=2) as ctxa_pool,
            tc.tile_pool(name="ps", bufs=3, space="PSUM") as ps,
        ):
            # ---- resident constants ----
            # DMA emission order is the schedule priority. The PE's first
            # dependency is et0 + w1t[0..], then the hb chain's hs chunks.
            w1t_sb = consts.tile([128, KF + KD, DIM], F16)
            et0 = encT_pool.tile([128, KF, 128], F16, tag="et")
            for q in range(4):
                nc.sync.dma_start(
                    out=et0[:, 6 * q : 6 * (q + 1), :],
                    in_=encT[0][:, 6 * q : 6 * (q + 1), :],
                )
            for k in range(4):
                nc.sync.dma_start(out=w1t_sb[:, k, :], in_=w1t[k])
            hst_sb = consts.tile([128, KD, nb], F16)
            for k in range(KD):
                nc.sync.dma_start(out=hst_sb[:, k, :], in_=hst[k])
            b1_sb = consts.tile([1, DIM], F16)
            nc.sync.dma_start(out=b1_sb, in_=b1r[:])
            w2b_sb = consts.tile([128, DIM], F16)
            nc.sync.dma_start(out=w2b_sb, in_=w2b[:])
            ones_sb = consts.tile([128, 128], F16)
            nc.sync.dma_start(out=ones_sb, in_=onesb[:])
            for k in range(KD):  # hs chunks: the hb stage needs them
                nc.sync.dma_start(out=w1t_sb[:, KF + k, :], in_=w1t[KF + k])
            for k in range(4, KF):
                nc.sync.dma_start(out=w1t_sb[:, k, :], in_=w1t[k])
            # tail-only constants declared here, loaded late (low priority)
            w3t_sb = consts.tile([128, KF, DIM], F16)
            b3_sb = consts.tile([nb, DIM], F32)

            negc_sb = consts.tile([128, 1], F32)
            nc.vector.memset(negc_sb, EXP_SHIFT)

            hb_sb = consts.tile([nb, DIM], F16)
            hbflat_sb = consts.tile([1, nb, DIM], F16)
            e_sb = consts.tile([128, nj], F32)
            lparts_sb = consts.tile([1, nb, j_tiles], F32)
            linv_sb = consts.tile([1, nb], F32)
            invl_sb = consts.tile([nb, 1], F32)
            ctxT_sb = consts.tile([128, KF, nb], F16)
            out_sb = consts.tile([nb, DIM], F32)

            def emit_hb_stage():
                # hb = hs @ W1h.T + b1 (per-batch bias rows). Own PSUM tag so
                # it can sit between row-tile 0's k-loops without stealing
                # their slots. Emitted after row-tile 0's main matmuls so the
                # PE doesn't idle waiting for the (later-loaded) hs chunks.
                for nh in range(2):
                    sl = ds(nh * 512, 512)
                    hbp = ps.tile([nb, 512], F32, tag="hb", bufs=1)
                    for k in range(KD):
                        nc.tensor.matmul(
                            hbp,
                            hst_sb[:, k, :],
                            w1t_sb[:, KF + k, sl],
                            start=(k == 0),
                            stop=False,
                        )
                    nc.tensor.matmul(
                        hbp,
                        ones_sb[0:1, 0:nb],
                        b1_sb[0:1, sl],
                        start=False,
                        stop=True,
                    )
                    nc.vector.tensor_copy(hb_sb[:, sl], hbp)
                # gather the per-batch bias rows onto partition 0 (matmul rhs
                # operands must start at partition 0)
                nc.sync.dma_start(out=hbflat_sb, in_=hb_sb)

            # ---- main loop ----
            # The ctx chain for row-tile j (w broadcast + DVE mul/reduce) is
            # emitted after row-tile j+1's matmuls so the PE never waits on
            # the tanh->e->exp->spread chain.
            ctx_accs = {}
            pending = None

            def _emit_bias_tanh(b, nh, hp, th):
                sl = ds(nh * 512, 512)
                # bias last so the group doesn't wait on the hb chain
                nc.tensor.matmul(
                    hp,
                    ones_sb[0:1, :],
                    hbflat_sb[0:1, b, sl],
                    start=False,
                    stop=True,
                )
                nc.scalar.activation(
                    th[:, sl], hp, mybir.ActivationFunctionType.Tanh
                )

            def emit_ctx_tail(state):
                b, j, et, wr = state
                ctx_acc = ctx_accs[b]
                # broadcast w across partitions via K=1 outer product
                wbp = ps.tile([128, 128], F32, tag="wb", bufs=2)
                nc.tensor.matmul(wbp, ones_sb[0:1, :], wr, start=True, stop=True)
                wb = wrow_pool.tile([128, 128], F16, tag="wb")
                nc.vector.tensor_copy(wb, wbp)
                # ctx_partial[f-chunk c] = sum_s wb[:, s] * et[:, c, s]
                pr = prod_pool.tile([128, KF, 128], F16)
                nc.vector.tensor_mul(pr, et, _bcast_free(wb[:], KF))
                cpart = ctxa_pool.tile([128, KF], F32, tag="cpart")
                nc.vector.tensor_reduce(
                    out=cpart,
                    in_=pr,
                    axis=mybir.AxisListType.X,
                    op=mybir.AluOpType.add,
                )
                if j == 0:
                    nc.vector.tensor_copy(ctx_acc, cpart)
                else:
                    nc.vector.tensor_add(ctx_acc, ctx_acc, cpart)
                if j == j_tiles - 1:
                    # ctxT column for this batch (f16 for the W3 matmuls)
                    nc.vector.tensor_copy(ctxT_sb[:, :, b], ctx_acc)

            for b in range(nb):
                ctx_acc_b = ctxa_pool.tile([128, KF], F32, tag="ctx_acc")
                ctx_accs[b] = ctx_acc_b
                for j in range(j_tiles):
                    jj = b * j_tiles + j
                    if jj == 0:
                        et = et0
                    else:
                        et = encT_pool.tile([128, KF, 128], F16, tag="et")
                        nc.sync.dma_start(out=et, in_=encT[jj])
                    # spread the w3t prefetch across the main loop
                    if jj >= min(4, nj - 1):
                        span = max(nj - min(4, nj - 1), 1)
                        pos = jj - min(4, nj - 1)
                        lo, hi = pos * KF // span, (pos + 1) * KF // span
                        for kk in range(lo, min(hi, KF)):
                            nc.sync.dma_start(out=w3t_sb[:, kk, :], in_=w3t[kk])
                    if jj == nj - 1:
                        nc.sync.dma_start(out=b3_sb, in_=b3b[:])
                    th = tanh_pool.tile([128, DIM], F16)
                    hps = []
                    for nh in range(2):
                        sl = ds(nh * 512, 512)
                        hp = ps.tile([128, 512], F32, tag="h")
                        hps.append(hp)
                        for k in range(KF):
                            nc.tensor.matmul(
                                hp,
                                et[:, k, :],
                                w1t_sb[:, k, sl],
                                start=(k == 0),
                                stop=False,
                            )
                        if jj > 0:
                            _emit_bias_tanh(b, nh, hp, th)
                    if jj == 0:
                        # row-tile 0: hb stage runs after the main k-loops so
                        # the PE starts on enc data, not on the hb chain
                        emit_hb_stage()
                        for nh in range(2):
                            _emit_bias_tanh(b, nh, hps[nh], th)
                    sc = scratch_pool.tile([128, DIM], F16)
                    nc.vector.scalar_tensor_tensor(
                        out=sc,
                        in0=th,
                        scalar=1.0,
                        in1=w2b_sb,
                        op0=mybir.AluOpType.mult,
                        op1=mybir.AluOpType.mult,
                        accum_out=e_sb[:, jj : jj + 1],
                    )
                    # w = exp(e-4) as a column, spread to a row via DMA
                    wc = wrow_pool.tile([128, 1], F16, tag="wc")
                    nc.scalar.activation(
                        wc,
                        e_sb[:, jj : jj + 1],
                        mybir.ActivationFunctionType.Exp,
                        bias=negc_sb,
                    )
                    wr = wrow_pool.tile([1, 128], F16)
                    nc.sync.dma_start(out=wr, in_=wc)
                    nc.vector.tensor_reduce(
                        out=lparts_sb[0:1, b, j : j + 1],
                        in_=wr,
                        axis=mybir.AxisListType.X,
                        op=mybir.AluOpType.add,
                    )
                    if pending is not None:
                        emit_ctx_tail(pending)
                    pending = (b, j, et, wr)
            emit_ctx_tail(pending)

            # ---- 1/l per batch, spread to a partition-column ----
            nc.vector.tensor_reduce(
                out=linv_sb,
                in_=lparts_sb,
                axis=mybir.AxisListType.X,
                op=mybir.AluOpType.add,
            )
            nc.vector.reciprocal(linv_sb, linv_sb)
            nc.sync.dma_start(out=invl_sb, in_=linv_sb[0:1, :])

            # ---- out = (ctx @ W3.T) * inv_l + b3 ----
            for nh in range(2):
                sl = ds(nh * 512, 512)
                wp = ps.tile([nb, 512], F32, tag="h")
                for k in range(KF):
                    nc.tensor.matmul(
                        wp,
                        ctxT_sb[:, k, :],
                        w3t_sb[:, k, sl],
                        start=(k == 0),
                        stop=(k == KF - 1),
                    )
                nc.vector.scalar_tensor_tensor(
                    out=out_sb[:, sl],
                    in0=wp,
                    scalar=invl_sb,
                    in1=b3_sb[:, sl],
                    op0=mybir.AluOpType.mult,
                    op1=mybir.AluOpType.add,
                )
            nc.sync.dma_start(out=out_d[:], in_=out_sb)

    _split_multiwaits(nc)
    return nc


def make_in_maps(hidden_state, encoder_outputs, W1, b1, w2, W3, b3, nb, j_tiles):
    """Shard + lay out the full inputs for each core. Returns list of dicts."""
    f16, f32 = np.float16, np.float32
    nj = nb * j_tiles
    s_core = j_tiles * 128

    w1t = np.ascontiguousarray(W1.T.reshape(KF + KD, 128, DIM)).astype(f16)
    w3t = np.ascontiguousarray(W3.T.reshape(KF, 128, DIM)).astype(f16)
    b1r = b1.reshape(1, DIM).astype(f16)
    w2b = np.ascontiguousarray(np.broadcast_to(w2.reshape(1, DIM), (128, DIM))).astype(
        f16
    )
    onesb = np.ones((128, 128), f16)
    b3b_full = np.ascontiguousarray(
        np.broadcast_to(b3.reshape(1, DIM), (nb, DIM))
    ).astype(f32)

    in_maps = []
    for i in range(N_CORES):
        bs = slice(i * nb, (i + 1) * nb)
        enc_c = encoder_outputs[bs, :s_core, :]  # (nb, s_core, F)
        e5 = enc_c.reshape(nb, j_tiles, 128, KF, 128)
        encT = np.ascontiguousarray(e5.transpose(0, 1, 4, 3, 2)).astype(f16)
        hs_c = hidden_state[bs]  # (nb, DIM)
        hst = np.ascontiguousarray(hs_c.T.reshape(KD, 128, nb)).astype(f16)
        in_maps.append(
            {
                "encT": encT.reshape(nj, 128, KF, 128),
                "w1t": w1t,
                "w3t": w3t,
                "hst": hst,
                "b1r": b1r,
                "w2b": w2b,
                "b3b": b3b_full,
                "onesb": onesb,
            }
        )
    return in_maps


_CACHE = {}


def run(hidden_state, encoder_outputs, W1, b1, w2, W3, b3, nb, j_tiles, trace=False):
    key = (nb, j_tiles)
    if key not in _CACHE:
        _CACHE[key] = build_bass(nb, j_tiles)
    nc = _CACHE[key]
    in_maps = make_in_maps(
        hidden_state, encoder_outputs, W1, b1, w2, W3, b3, nb, j_tiles
    )
    res = bass_utils.run_bass_kernel_spmd(
        nc, in_maps, list(range(N_CORES)), trace=trace
    )
    out = np.concatenate([res.results[i]["out"] for i in range(N_CORES)], axis=0)
    return out.astype(np.float32), res


def kernel(hidden_state, encoder_outputs, W1, b1, w2, W3, b3):
    hidden_state = np.asarray(hidden_state, dtype=np.float32)
    encoder_outputs = np.asarray(encoder_outputs, dtype=np.float32)
    W1 = np.asarray(W1, dtype=np.float32)
    b1 = np.asarray(b1, dtype=np.float32)
    w2 = np.asarray(w2, dtype=np.float32)
    W3 = np.asarray(W3, dtype=np.float32)
    b3 = np.asarray(b3, dtype=np.float32)
    out, _ = run(hidden_state, encoder_outputs, W1, b1, w2, W3, b3, nb=4, j_tiles=8)
    return out


# revision 31
# speedup vs baseline: 1.0132x; 1.0007x over previous
"""Trainium2 Bass kernel for the aux-attention module.

reference (per batch b):
    inputs = concat([enc[b], broadcast(hs[b])], -1)          # (S, 4096)
    hidden = tanh(inputs @ W1.T + b1)                        # (S, 1024)
    e      = hidden @ w2.T                                   # (S,)
    alpha  = softmax(e)
    ctx    = alpha @ enc[b]                                  # (3072,)
    out[b] = ctx @ W3.T + b3                                 # (1024,)

Strategy: data-parallel over batch (4 batches/core x 8 cores), weights
replicated. All PE matmuls in fp16 (fp32 PSUM accumulation). Softmax without
max-subtraction: w = exp(e - 4) unnormalized (e is O(1) for this model), the
1/sum(w) normalization is folded into the final output scaling.

Per core, per 128-row tile (single pass over enc, f-major layout from host):
  - hidden = tanh(enc_tile @ W1e.T + hb) on PE (25 N=512 matmuls) + ACT
  - e column via one fused DVE multiply+accumulate against broadcast w2
  - w = exp(e-4) (ACT), column -> row via a tiny cross-partition DMA,
    broadcast across partitions (K=1 matmul outer product), then
    ctx_partial[f-chunk] = sum_s w[s]*enc[f, s] as a DVE multiply +
    per-chunk reduce on the same f-major tile already in SBUF (no second
    HBM read of enc). This chain is emitted one row-tile behind the
    matmul stream so the PE never waits on it.
Tail: inv_l via reduce+reciprocal, out = (ctxT @ W3.T) * inv_l + b3.
"""

import numpy as np

try:  # persistent compile cache: repeated runs skip the walrus compile
    import jax

    jax.config.update("jax_compilation_cache_dir", "/tmp/jax_neff_cache")
    jax.config.update("jax_persistent_cache_min_compile_time_secs", 1.0)
except Exception:
    pass

import concourse.bass as bass
import concourse.tile as tile
from concourse import mybir
from concourse.bass import ds
from concourse import bass_utils

# ---------------------------------------------------------------------------
# Walrus in this container caps sync waits per instruction (one; two for
# EventSemaphore). Tile's tail drain carries one wait per live semaphore and
# Tile occasionally leaks multi-wait instructions; split extras onto cheap
# carriers.
from concourse import tile as _tile_mod
from concourse import mybir as _mybir


def _patched_drain_and_barrier(self, tick_clock, wait_clock):
    nc = self.nc
    drain_inst = nc.sync.drain()
    wait_clock.add_sem_waits(
        drain_inst.ins, _tile_mod.ScopedClock({None: tick_clock.global_clock})
    )
    si = drain_inst.ins.sync_info
    waits = list(si.on_wait) if si is not None else []
    if len(waits) > 1:
        drain_inst.ins.sync_info = _mybir.SyncInfo(on_update=[], on_wait=waits[:1])
        for w in waits[1:]:
            extra = nc.sync.nop(nofuse=True, hint="drain_wait_split")
            extra.ins.sync_info = _mybir.SyncInfo(on_update=[], on_wait=[w])
    nc.all_engine_barrier()
    assert self.sems is not None
    popped = nc._tile_sem_poison_stack.pop()
    assert popped is self._sem_poison
    nc.clear_and_free_semaphores(list(self.sems.allocated().values()))
    nc.all_engine_barrier()


_tile_mod.TileContext._drain_and_barrier = _patched_drain_and_barrier


def _split_multiwaits(nc):
    for fn in nc.m.functions:
        for blk in fn.blocks:
            out, changed = [], False
            for inst in list(blk.instructions):
                si = inst.sync_info
                waits = list(si.on_wait) if si is not None else []
                cap = 2 if inst.opcode == "EventSemaphore" else 1
                if len(waits) > cap:
                    changed = True
                    for idx, w in enumerate(waits[:-cap]):
                        nop = _mybir.InstNoOp(
                            name=f"{inst.name}-wsplit{idx}", ins=[], outs=[]
                        )
                        nop.engine = inst.engine
                        nop.sync_info = _mybir.SyncInfo(on_update=[], on_wait=[w])
                        out.append(nop)
                    inst.sync_info = _mybir.SyncInfo(
                        on_update=list(si.on_update), on_wait=waits[-cap:]
                    )
                out.append(inst)
            if changed:
                blk.instructions = out


# ---------------------------------------------------------------------------

F16 = mybir.dt.float16
F32 = mybir.dt.float32

N_CORES = 8
B, S, DIM, F = 32, 1024, 1024, 3072  # F = enc feature dim; DIM = model dim
KF = F // 128  # 24 enc k-tiles
KD = DIM // 128  # 8 hs k-tiles
EXP_SHIFT = -4.0  # w = exp(e + EXP_SHIFT); e is O(1), shift keeps fp16 safe


def _bcast_free(ap, n, at=1):
    """Insert a step-0 (broadcast) free dim of size n at position `at`."""
    aps = list(ap.ap)
    aps.insert(at, [0, n])
    return bass.AP(tensor=ap.tensor, offset=ap.offset, ap=aps)


def build_bass(nb, j_tiles):
    """nb batches per core, j_tiles row-tiles of 128 per batch."""
    nj = nb * j_tiles
    nc = bass.Bass()
    encT = nc.declare_dram_parameter("encT", [nj, 128, KF, 128], F16, isOutput=False)
    w1t = nc.declare_dram_parameter("w1t", [KF + KD, 128, DIM], F16, isOutput=False)
    w3t = nc.declare_dram_parameter("w3t", [KF, 128, DIM], F16, isOutput=False)
    hst = nc.declare_dram_parameter("hst", [KD, 128, nb], F16, isOutput=False)
    b1r = nc.declare_dram_parameter("b1r", [1, DIM], F16, isOutput=False)
    w2b = nc.declare_dram_parameter("w2b", [128, DIM], F16, isOutput=False)
    b3b = nc.declare_dram_parameter("b3b", [nb, DIM], F32, isOutput=False)
    onesb = nc.declare_dram_parameter("onesb", [128, 128], F16, isOutput=False)
    out_d = nc.declare_dram_parameter("out", [nb, DIM], F32, isOutput=True)

    with tile.TileContext(nc) as tc:
        with (
            tc.tile_pool(name="consts", bufs=1) as consts,
            tc.tile_pool(name="encT", bufs=5) as encT_pool,
            tc.tile_pool(name="tanh", bufs=3) as tanh_pool,
            tc.tile_pool(name="scratch", bufs=1) as scratch_pool,
            tc.tile_pool(name="prod", bufs=2) as prod_pool,
            tc.tile_pool(name="wrow", bufs=3) as wrow_pool,
            tc.tile_pool(name="ctxa", bufs=2) as ctxa_pool,
            tc.tile_pool(name="ps", bufs=4, space="PSUM") as ps,
        ):
            # ---- resident constants ----
            # DMA emission order is the schedule priority. The PE's first
            # dependency is et0 + w1t[0..], then the hb chain's hs chunks.
            w1t_sb = consts.tile([128, KF + KD, DIM], F16)
            et0 = encT_pool.tile([128, KF, 128], F16, tag="et")
            for q in range(4):
                nc.sync.dma_start(
                    out=et0[:, 6 * q : 6 * (q + 1), :],
                    in_=encT[0][:, 6 * q : 6 * (q + 1), :],
                )
            for k in range(4):
                nc.sync.dma_start(out=w1t_sb[:, k, :], in_=w1t[k])
            hst_sb = consts.tile([128, KD, nb], F16)
            for k in range(KD):
                nc.sync.dma_start(out=hst_sb[:, k, :], in_=hst[k])
            b1_sb = consts.tile([1, DIM], F16)
            nc.sync.dma_start(out=b1_sb, in_=b1r[:])
            w2b_sb = consts.tile([128, DIM], F16)
            nc.sync.dma_start(out=w2b_sb, in_=w2b[:])
            ones_sb = consts.tile([128, 128], F16)
            nc.sync.dma_start(out=ones_sb, in_=onesb[:])
            for k in range(KD):  # hs chunks: the hb stage needs them
                nc.sync.dma_start(out=w1t_sb[:, KF + k, :], in_=w1t[KF + k])
            for k in range(4, KF):
                nc.sync.dma_start(out=w1t_sb[:, k, :], in_=w1t[k])
            # tail-only constants declared here, loaded late (low priority)
            w3t_sb = consts.tile([128, KF, DIM], F16)
            b3_sb = consts.tile([nb, DIM], F32)

            negc_sb = consts.tile([128, 1], F32)
            nc.vector.memset(negc_sb, EXP_SHIFT)

            hb_sb = consts.tile([nb, DIM], F16)
            hbflat_sb = consts.tile([1, nb, DIM], F16)
            e_sb = consts.tile([128, nj], F32)
            lparts_sb = consts.tile([1, nb, j_tiles], F32)
            linv_sb = consts.tile([1, nb], F32)
            invl_sb = consts.tile([nb, 1], F32)
            ctxT_sb = consts.tile([128, KF, nb], F16)
            out_sb = consts.tile([nb, DIM], F32)

            def emit_hb_stage():
                # hb = hs @ W1h.T + b1 (per-batch bias rows). Own PSUM tag so
                # it can sit between row-tile 0's k-loops without stealing
                # their slots. Emitted after row-tile 0's main matmuls so the
                # PE doesn't idle waiting for the (later-loaded) hs chunks.
                for nh in range(2):
                    sl = ds(nh * 512, 512)
                    hbp = ps.tile([nb, 512], F32, tag="hb", bufs=1)
                    for k in range(KD):
                        nc.tensor.matmul(
                            hbp,
                            hst_sb[:, k, :],
                            w1t_sb[:, KF + k, sl],
                            start=(k == 0),
                            stop=False,
                        )
                    nc.tensor.matmul(
                        hbp,
                        ones_sb[0:1, 0:nb],
                        b1_sb[0:1, sl],
                        start=False,
                        stop=True,
                    )
                    nc.vector.tensor_copy(hb_sb[:, sl], hbp)
                # gather the per-batch bias rows onto partition 0 (matmul rhs
                # operands must start at partition 0)
                nc.sync.dma_start(out=hbflat_sb, in_=hb_sb)

            # ---- main loop ----
            # The ctx chain for row-tile j (w broadcast + DVE mul/reduce) is
            # emitted after row-tile j+1's matmuls so the PE never waits on
            # the tanh->e->exp->spread chain.
            ctx_accs = {}
            pending = None

            def _emit_bias_tanh(b, nh, hp, th):
                sl = ds(nh * 512, 512)
                # bias last so the group doesn't wait on the hb chain
                nc.tensor.matmul(
                    hp,
                    ones_sb[0:1, :],
                    hbflat_sb[0:1, b, sl],
                    start=False,
                    stop=True,
                )
                nc.scalar.activation(
                    th[:, sl], hp, mybir.ActivationFunctionType.Tanh
                )

            def emit_ctx_tail(state):
                b, j, et, wr = state
                ctx_acc = ctx_accs[b]
                # broadcast w across partitions via K=1 outer product
                wbp = ps.tile([128, 128], F32, tag="wb", bufs=2)
                nc.tensor.matmul(wbp, ones_sb[0:1, :], wr, start=True, stop=True)
                wb = wrow_pool.tile([128, 128], F16, tag="wb")
                nc.vector.tensor_copy(wb, wbp)
                # ctx_partial[f-chunk c] = sum_s wb[:, s] * et[:, c, s]
                pr = prod_pool.tile([128, KF, 128], F16)
                nc.vector.tensor_mul(pr, et, _bcast_free(wb[:], KF))
                cpart = ctxa_pool.tile([128, KF], F32, tag="cpart")
                nc.vector.tensor_reduce(
                    out=cpart,
                    in_=pr,
                    axis=mybir.AxisListType.X,
                    op=mybir.AluOpType.add,
                )
                if j == 0:
                    nc.vector.tensor_copy(ctx_acc, cpart)
                else:
                    nc.vector.tensor_add(ctx_acc, ctx_acc, cpart)
                if j == j_tiles - 1:
                    # ctxT column for this batch (f16 for the W3 matmuls)
                    nc.vector.tensor_copy(ctxT_sb[:, :, b], ctx_acc)

            for b in range(nb):
                ctx_acc_b = ctxa_pool.tile([128, KF], F32, tag="ctx_acc")
                ctx_accs[b] = ctx_acc_b
                for j in range(j_tiles):
                    jj = b * j_tiles + j
                    if jj == 0:
                        et = et0
                    else:
                        et = encT_pool.tile([128, KF, 128], F16, tag="et")
                        nc.sync.dma_start(out=et, in_=encT[jj])
                    # spread the w3t prefetch across the main loop
                    if jj >= min(4, nj - 1):
                        span = max(nj - min(4, nj - 1), 1)
                        pos = jj - min(4, nj - 1)
                        lo, hi = pos * KF // span, (pos + 1) * KF // span
                        for kk in range(lo, min(hi, KF)):
                            nc.sync.dma_start(out=w3t_sb[:, kk, :], in_=w3t[kk])
                    if jj == nj - 1:
                        nc.sync.dma_start(out=b3_sb, in_=b3b[:])
                    th = tanh_pool.tile([128, DIM], F16)
                    hps = []
                    for nh in range(2):
                        sl = ds(nh * 512, 512)
                        hp = ps.tile([128, 512], F32, tag="h")
                        hps.append(hp)
                        for k in range(KF):
                            nc.tensor.matmul(
                                hp,
                                et[:, k, :],
                                w1t_sb[:, k, sl],
                                start=(k == 0),
                                stop=False,
                            )
                        if jj > 0:
                            _emit_bias_tanh(b, nh, hp, th)
                    if jj == 0:
                        # row-tile 0: hb stage runs after the main k-loops so
                        # the PE starts on enc data, not on the hb chain
                        emit_hb_stage()
                        for nh in range(2):
                            _emit_bias_tanh(b, nh, hps[nh], th)
                    sc = scratch_pool.tile([128, DIM], F16)
                    nc.vector.scalar_tensor_tensor(
                        out=sc,
                        in0=th,
                        scalar=1.0,
                        in1=w2b_sb,
                        op0=mybir.AluOpType.mult,
                        op1=mybir.AluOpType.mult,
                        accum_out=e_sb[:, jj : jj + 1],
                    )
                    # w = exp(e-4) as a column, spread to a row via DMA
                    wc = wrow_pool.tile([128, 1], F16, tag="wc")
                    nc.scalar.activation(
                        wc,
                        e_sb[:, jj : jj + 1],
                        mybir.ActivationFunctionType.Exp,
                        bias=negc_sb,
                    )
                    wr = wrow_pool.tile([1, 128], F16)
                    nc.sync.dma_start(out=wr, in_=wc)
                    nc.vector.tensor_reduce(
                        out=lparts_sb[0:1, b, j : j + 1],
                        in_=wr,
                        axis=mybir.AxisListType.X,
                        op=mybir.AluOpType.add,
                    )
                    if pending is not None:
                        emit_ctx_tail(pending)
                    pending = (b, j, et, wr)
            emit_ctx_tail(pending)

            # ---- 1/l per batch, spread to a partition-column ----
            nc.vector.tensor_reduce(
                out=linv_sb,
                in_=lparts_sb,
                axis=mybir.AxisListType.X,
                op=mybir.AluOpType.add,
            )
            nc.vector.reciprocal(linv_sb, linv_sb)
            nc.sync.dma_start(out=invl_sb, in_=linv_sb[0:1, :])

            # ---- out = (ctx @ W3.T) * inv_l + b3 ----
            for nh in range(2):
                sl = ds(nh * 512, 512)
                wp = ps.tile([nb, 512], F32, tag="h")
                for k in range(KF):
                    nc.tensor.matmul(
                        wp,
                        ctxT_sb[:, k, :],
                        w3t_sb[:, k, sl],
                        start=(k == 0),
                        stop=(k == KF - 1),
                    )
                nc.vector.scalar_tensor_tensor(
                    out=out_sb[:, sl],
                    in0=wp,
                    scalar=invl_sb,
                    in1=b3_sb[:, sl],
                    op0=mybir.AluOpType.mult,
                    op1=mybir.AluOpType.add,
                )
            nc.sync.dma_start(out=out_d[:], in_=out_sb)

    _split_multiwaits(nc)
    return nc


def make_in_maps(hidden_state, encoder_outputs, W1, b1, w2, W3, b3, nb, j_tiles):
    """Shard + lay out the full inputs for each core. Returns list of dicts."""
    f16, f32 = np.float16, np.float32
    nj = nb * j_tiles
    s_core = j_tiles * 128

    w1t = np.ascontiguousarray(W1.T.reshape(KF + KD, 128, DIM)).astype(f16)
    w3t = np.ascontiguousarray(W3.T.reshape(KF, 128, DIM)).astype(f16)
    b1r = b1.reshape(1, DIM).astype(f16)
    w2b = np.ascontiguousarray(np.broadcast_to(w2.reshape(1, DIM), (128, DIM))).astype(
        f16
    )
    onesb = np.ones((128, 128), f16)
    b3b_full = np.ascontiguousarray(
        np.broadcast_to(b3.reshape(1, DIM), (nb, DIM))
    ).astype(f32)

    in_maps = []
    for i in range(N_CORES):
        bs = slice(i * nb, (i + 1) * nb)
        enc_c = encoder_outputs[bs, :s_core, :]  # (nb, s_core, F)
        e5 = enc_c.reshape(nb, j_tiles, 128, KF, 128)
        encT = np.ascontiguousarray(e5.transpose(0, 1, 4, 3, 2)).astype(f16)
        hs_c = hidden_state[bs]  # (nb, DIM)
        hst = np.ascontiguousarray(hs_c.T.reshape(KD, 128, nb)).astype(f16)
        in_maps.append(
            {
                "encT": encT.reshape(nj, 128, KF, 128),
                "w1t": w1t,
                "w3t": w3t,
                "hst": hst,
                "b1r": b1r,
                "w2b": w2b,
                "b3b": b3b_full,
                "onesb": onesb,
            }
        )
    return in_maps


_CACHE = {}


def run(hidden_state, encoder_outputs, W1, b1, w2, W3, b3, nb, j_tiles, trace=False):
    key = (nb, j_tiles)
    if key not in _CACHE:
        _CACHE[key] = build_bass(nb, j_tiles)
    nc = _CACHE[key]
    in_maps = make_in_maps(
        hidden_state, encoder_outputs, W1, b1, w2, W3, b3, nb, j_tiles
    )
    res = bass_utils.run_bass_kernel_spmd(
        nc, in_maps, list(range(N_CORES)), trace=trace
    )
    out = np.concatenate([res.results[i]["out"] for i in range(N_CORES)], axis=0)
    return out.astype(np.float32), res


def kernel(hidden_state, encoder_outputs, W1, b1, w2, W3, b3):
    hidden_state = np.asarray(hidden_state, dtype=np.float32)
    encoder_outputs = np.asarray(encoder_outputs, dtype=np.float32)
    W1 = np.asarray(W1, dtype=np.float32)
    b1 = np.asarray(b1, dtype=np.float32)
    w2 = np.asarray(w2, dtype=np.float32)
    W3 = np.asarray(W3, dtype=np.float32)
    b3 = np.asarray(b3, dtype=np.float32)
    out, _ = run(hidden_state, encoder_outputs, W1, b1, w2, W3, b3, nb=4, j_tiles=8)
    return out
